# revision 35
# baseline (speedup 1.0000x reference)
"""AdaAtt attention block on 8 TRN2 NeuronCores — v2.

Data-parallel over batch (16/core), weights replicated. Rebuilt around
three findings from the v1 trace: DMA was descriptor-generation-bound
(34.7k small packets, MBU 24%), the PE was LDWEIGHTS/instruction-count
bound (868 matmul+ldw pairs), and DVE burned 35us on per-(b,chunk)
broadcast adds.

Changes:
- Every DMA is SBUF-tile-exact with >=2KB contiguous per-partition runs
  (hundreds of descriptors total instead of 34.7k).
- fp8e4 storage for conv_feat, conv_feat_embed, W_fr, W_fre, W_hoe, W_a
  (numpy-emulated rel-err 5.7e-3 vs the 2e-2 budget); W_ho and W_h stay
  bf16 (fp8 there alone costs 3e-2). Mixed fp8xbf16 matmuls are legal on
  the PE, and fp8 stationaries get the fast weight load path.
- Dense layers accumulate all 8 output chunks in ONE psum bank with a
  single start/stop group; bias enters via K=1 ones-trick matmuls so
  each layer needs one activation instruction.
- The hoe broadcast-add is one DVE tensor_tensor per batch (free-dim
  stride-0 broadcast), tanh is one big ACT op per batch writing fp8.
- Scores use DoubleRow fp8 matmuls (K=256 per instruction, 4/batch)
  into a [16,196] psum tile, so softmax runs batched in natural layout
  with no transposes of scores.
- The l=0 fake-region slot is injected into the conv value tile (row 0
  of the first l-chunk) with a tiny SBUF->SBUF DMA, so visAtt picks it
  up for free; softmax/visAtt run per 4-batch group to shrink the tail.
"""

import sys

if "/opt/trn_rl_repo" not in sys.path:
    sys.path.insert(0, "/opt/trn_rl_repo")

import numpy as np

import concourse.bass as bass
import concourse.tile as tile
from concourse import mybir
from concourse import bacc
from concourse.bass_utils import run_bass_kernel_spmd
from concourse.masks import make_identity

# ---------------------------------------------------------------------------

B, L, D = 128, 196, 1024
N_CORES = 8
S = B // N_CORES          # batches per core
CH = D // 128             # 128-wide chunks of D
LC = 98                   # conv l-chunk rows; c0 holds l0-slot + l=1..98
LP = 104                  # ha l-chunk pitch (98 padded so kt-stride is 16n)
G = 4                     # batches per softmax/visAtt group
NG = S // G

F32 = mybir.dt.float32
BF16 = mybir.dt.bfloat16
F8 = mybir.dt.float8e4

ACTF = mybir.ActivationFunctionType
ALU = mybir.AluOpType
DR = mybir.MatmulPerfMode.DoubleRow


def build_nc(stage: int = 9) -> bass.Bass:
    nc = bacc.Bacc()

    def param(name, shape, dt, out=False):
        return nc.declare_dram_parameter(name, list(shape), dt, isOutput=out)

    xfr_d = param("xfr_T", (128, CH, S), BF16)
    xho_d = param("xho_T", (128, CH, S), BF16)
    w_d = {
        "ho": param("w_ho", (128, CH, D), BF16),
        "hoe": param("w_hoe", (128, CH, D), F8),
        "fr": param("w_fr", (128, CH, D), F8),
        "fre": param("w_fre", (128, CH, D), F8),
        "h": param("w_h", (128, CH, D), BF16),
    }
    bias_d = param("bias_row", (1, 5, D), BF16)
    wa_d = param("wa8", (128, CH, 16), F8)
    cfe_d = param("cfe8", (128, S, CH, L), F8)
    conv_d = param("conv8", (LC + 1, S, 2, D), F8)
    out_d = param("out", (128, CH, S), F32, out=True)

    LI = {"fr": 0, "fre": 1, "ho": 2, "hoe": 3, "h": 4}

    with tile.TileContext(nc) as tc:
        with (
            tc.tile_pool(name="singles", bufs=1) as singles,
            tc.tile_pool(name="w16p", bufs=4) as w16p,
            tc.tile_pool(name="w8p", bufs=6) as w8p,
            tc.tile_pool(name="acts", bufs=1) as acts,
            tc.tile_pool(name="cfep", bufs=NG) as cfep,
            tc.tile_pool(name="hap", bufs=NG) as hap,
            tc.tile_pool(name="convp", bufs=NG) as convp,
            tc.tile_pool(name="sumr", bufs=3) as sumr,
            tc.tile_pool(name="pic", bufs=NG) as pic,
            tc.tile_pool(name="mm_ps", bufs=2, space="PSUM") as mm_ps,
            tc.tile_pool(name="aux_ps", bufs=1, space="PSUM") as aux_ps,
            tc.tile_pool(name="big_ps", bufs=1, space="PSUM") as big_ps,
        ):
            # --- constants / small loads -----------------------------------
            xho_t = singles.tile([128, CH, S], BF16)
            nc.sync.dma_start(out=xho_t, in_=xho_d[:, :, :])
            bias_t = singles.tile([1, 5, D], BF16)
            nc.sync.dma_start(out=bias_t, in_=bias_d[:, :, :])

            def wload(lname, wpool, wdt, rings=(None, None)):
                w_c = []
                for kc in range(2):
                    t = wpool.tile([128, 4, D], wdt, tag=f"w_{wdt}")
                    ring = rings[kc] or nc.sync
                    ring.dma_start(
                        out=t, in_=w_d[lname][:, 4 * kc:4 * kc + 4, :]
                    )
                    w_c.append(t)
                return w_c

            # chunk 0 on the sync hw queue, chunk 1 on the scalar hw queue:
            # two queues pull in parallel (one queue caps at ~200 GB/s)
            w_ho_c = wload("ho", w16p, BF16, (nc.sync, nc.scalar))
            w_hoe_c = wload("hoe", w8p, F8, (nc.sync, nc.scalar))

            ones_t = singles.tile([1, S], BF16)
            nc.vector.memset(ones_t, 1.0)
            id_bf = singles.tile([128, 128], BF16)
            make_identity(nc, id_bf)
            id_f32 = singles.tile([128, 128], F32)
            make_identity(nc, id_f32)

            # --- dense layers (W stationary, one psum bank per layer) ------
            def dense(lname, rhs_sb, func, out_dt, w_c):
                ps = mm_ps.tile([128, CH, S], F32, tag="mm")
                li = LI[lname]
                for o in range(CH):
                    nc.tensor.matmul(
                        ps[:, o, :],
                        lhsT=bias_t[0:1, li, o * 128:(o + 1) * 128],
                        rhs=ones_t,
                        start=(o == 0), stop=False,
                        tile_position=(0, 0),
                    )
                for kc in range(2):
                    for k in range(4):
                        for o in range(CH):
                            last = kc == 1 and k == 3 and o == CH - 1
                            nc.tensor.matmul(
                                ps[:, o, :],
                                lhsT=w_c[kc][:, k, o * 128:(o + 1) * 128],
                                rhs=rhs_sb[:, 4 * kc + k, :],
                                start=False, stop=last,
                            )
                out_sb = acts.tile([128, CH, S], out_dt, tag=f"act_{lname}")
                nc.scalar.activation(
                    out=out_sb.rearrange("p c b -> p (c b)"),
                    in_=ps.rearrange("p c b -> p (c b)"),
                    func=func,
                )
                return out_sb

            # --- stream DMAs --------------------------------------------
            # Three hw queues. The scalar queue gets ONLY 6 early triggers
            # (a deeper burst overflows the HWDGE ring and the blocked
            # trigger wedges the ACT engine behind it). conv + w_h ride
            # the gpsimd/SWDGE queue, gated behind a pre-write that waits
            # for ho_t so they cannot steal bus bandwidth from the
            # critical prefix. Everything else (and all dep-gated DMAs)
            # lives on the compute-free sync queue.
            def cfe_load(q, ring):
                t = cfep.tile(
                    [128, G, CH, L], F8, tag="cfe", name=f"cfe_{q}"
                )
                ring.dma_start(out=t, in_=cfe_d[:, G * q:G * q + G, :, :])
                return t

            cfe_q = [None] * NG
            cfe_q[0] = cfe_load(0, nc.sync)
            wa_t = singles.tile([128, CH, 16], F8)
            nc.sync.dma_start(out=wa_t, in_=wa_d[:, :, :])
            xfr_t = singles.tile([128, CH, S], BF16)
            nc.sync.dma_start(out=xfr_t, in_=xfr_d[:, :, :])
            cfe_q[1] = cfe_load(1, nc.scalar)
            cfe_q[2] = cfe_load(2, nc.sync)
            cfe_q[3] = cfe_load(3, nc.scalar)
            w_fr_c = wload("fr", w8p, F8, (nc.sync, nc.scalar))
            w_fre_c = wload("fre", w8p, F8, (nc.sync, nc.scalar))

            ho_t = dense("ho", xho_t, ACTF.Tanh, BF16, w_ho_c)

            conv_q = []
            for q in range(NG):
                t = convp.tile(
                    [LC + 1, G, 2, D], F8, tag="conv", name=f"conv_{q}"
                )
                conv_q.append(t)
            w_h_c = [
                w16p.tile(
                    [128, 4, D], BF16, tag=f"w_{BF16}", name=f"w_h_{kc}"
                )
                for kc in range(2)
            ]
            gate_src = ho_t[0:1, 0, 0:4]
            pool_order = [
                (conv_q[0], conv_d[:, 0:G, :, :]),
                (conv_q[1], conv_d[:, G:2 * G, :, :]),
                (w_h_c[0], w_d["h"][:, 0:4, :]),
                (w_h_c[1], w_d["h"][:, 4:8, :]),
                (conv_q[2], conv_d[:, 2 * G:3 * G, :, :]),
                (conv_q[3], conv_d[:, 3 * G:4 * G, :, :]),
            ]
            for dst, src in pool_order:
                head = (
                    dst[0:1, 0, 0, 0:4] if dst.ndim == 4 else dst[0:1, 0, 0:4]
                )
                nc.gpsimd.tensor_copy(head, gate_src)
                nc.gpsimd.dma_start(out=dst, in_=src)

            hoe_t = dense("hoe", ho_t, ACTF.Identity, BF16, w_hoe_c)

            frst = {}

            def emit_fr_chain():
                # emitted mid score-loop so the ACT/PE queue order matches
                # weight-arrival order (fr/fre ACTs would otherwise wedge
                # the tanh pipeline behind the w_fr DMA)
                fr_t = dense("fr", xfr_t, ACTF.Relu, BF16, w_fr_c)
                fre_t = dense("fre", fr_t, ACTF.Identity, BF16, w_fre_c)

                # --- fr -> natural fp8, inject as l=0 row of conv tiles --------
                frn_ps = aux_ps.tile([16, CH, 128], BF16, tag="sng", bufs=2)
                for c in range(CH):
                    nc.tensor.transpose(frn_ps[:, c, :], fr_t[:, c, :], id_bf)
                fr_nat8 = singles.tile([16, CH, 128], F8)
                nc.scalar.activation(
                    out=fr_nat8.rearrange("b c p -> b (c p)"),
                    in_=frn_ps.rearrange("b c p -> b (c p)"),
                    func=ACTF.Copy,
                )
                for q in range(NG):
                    nc.sync.dma_start(
                        out=conv_q[q][0:1, :, 0, :],
                        in_=fr_nat8[G * q:G * q + G, :, :],
                    )

                # --- l=0 slot score: tanh(fr_e + ho_e) . wa --------------------
                sum0 = acts.tile([128, CH, S], BF16, tag="sum0")
                nc.vector.tensor_add(sum0, fre_t, hoe_t)
                ha0 = acts.tile([128, CH, S], BF16, tag="ha0")
                nc.scalar.activation(
                    out=ha0.rearrange("p c b -> p (c b)"),
                    in_=sum0.rearrange("p c b -> p (c b)"),
                    func=ACTF.Tanh,
                )
                sc0_ps = mm_ps.tile([1, S], F32, tag="mm")
                for c in range(CH):
                    nc.tensor.matmul(
                        sc0_ps,
                        lhsT=wa_t[:, c, 0:1],
                        rhs=ha0[:, c, :],
                        start=(c == 0), stop=(c == CH - 1),
                        tile_position=(0, 0),
                    )
                sc0_sb = singles.tile([1, S], F32)
                nc.vector.tensor_copy(sc0_sb, sc0_ps)
                frst["fr"], frst["fre"] = fr_t, fre_t
                frst["sc0"] = sc0_sb

            # --- per-batch: bcast add -> tanh -> DoubleRow score mms -------
            # sc_cols[lp, c, b] = score for l = 1 + c*98 + lp
            sc_cols = big_ps.tile([LC, 2, S], F32, tag="sc")
            ha_q = [
                hap.tile([128, G, CH, 2, LP], F8, tag="ha", name=f"ha_{q}")
                for q in range(NG)
            ]
            GP_B = {5, 9}   # late-ish adds offloaded to the GpSimd engine
            for b in range(S):
                q, j = b // G, b % G
                eng = nc.gpsimd if b in GP_B else nc.vector
                sum_t = sumr.tile([128, CH, L], BF16, tag="sum", bufs=4)
                eng.tensor_tensor(
                    sum_t,
                    cfe_q[q][:, j, :, :],
                    hoe_t[:, :, b:b + 1].broadcast_to([128, CH, L]),
                    op=ALU.add,
                )
                nc.scalar.activation(
                    out=ha_q[q][:, j, :, :, 0:LC],
                    in_=sum_t.rearrange("p c (two l) -> p c two l", two=2),
                    func=ACTF.Tanh,
                )
                for c in range(2):
                    for sp in range(4):
                        nc.tensor.matmul(
                            sc_cols[:, c, b:b + 1],
                            lhsT=ha_q[q][:, j, 2 * sp:2 * sp + 2, c, 0:LC],
                            rhs=wa_t[:, 2 * sp:2 * sp + 2, 0:1],
                            start=(sp == 0), stop=(sp == 3),
                            perf_mode=DR,
                        )
                if b == 6:
                    emit_fr_chain()

            # --- per-group: softmax + pi transposes + visAtt ---------------
            va_ps = big_ps.tile([128, CH, S], F32, tag="va")
            first_va = [True]
            for g in range(NG):
                sl = slice(G * g, G * g + G)
                scg = sumr.tile([LC, 2, G], F32, tag="scg", bufs=2)
                nc.scalar.activation(
                    out=scg, in_=sc_cols[:, :, sl], func=ACTF.Copy
                )
                sng = aux_ps.tile([G, 1 + L], F32, tag="sng", bufs=2)
                nc.tensor.transpose(
                    sng[:, 0:1], frst["sc0"][0:1, sl], id_f32[0:1, 0:1]
                )
                for c in range(2):
                    nc.tensor.transpose(
                        sng[:, 1 + c * LC:1 + (c + 1) * LC],
                        scg[:, c, :], id_f32[0:LC, 0:LC],
                    )
                neg_mx = sumr.tile([G, 1], F32, tag="negmx", bufs=2)
                nc.vector.tensor_reduce(
                    out=neg_mx, in_=sng,
                    axis=mybir.AxisListType.X, op=ALU.max, negate=True,
                )
                exp_t = sumr.tile([G, 1 + L], F32, tag="exp", bufs=2)
                nc.scalar.activation(
                    out=exp_t, in_=sng,
                    func=ACTF.Exp, bias=neg_mx, scale=1.0,
                )
                ssum = sumr.tile([G, 1], F32, tag="ssum", bufs=2)
                nc.vector.tensor_reduce(
                    out=ssum, in_=exp_t,
                    axis=mybir.AxisListType.X, op=ALU.add,
                )
                rsum = sumr.tile([G, 1], F32, tag="rsum", bufs=2)
                nc.vector.reciprocal(rsum, ssum)
                pi_n = sumr.tile([G, 1 + L], BF16, tag="pi", bufs=2)
                nc.vector.tensor_scalar_mul(pi_n, exp_t, rsum)

                pi_cols = pic.tile([LC + 1, 2, G], BF16, tag="pic")
                tpg = aux_ps.tile([LC + 1, 2, G], BF16, tag="tpg", bufs=1)
                nc.tensor.transpose(
                    tpg[:, 0, :], pi_n[:, 0:LC + 1], id_bf[0:G, 0:G]
                )
                nc.tensor.transpose(
                    tpg[0:LC, 1, :], pi_n[:, LC + 1:1 + L], id_bf[0:G, 0:G]
                )
                nc.vector.tensor_copy(pi_cols[:, 0, :], tpg[:, 0, :])
                nc.vector.tensor_copy(pi_cols[0:LC, 1, :], tpg[0:LC, 1, :])

                for j in range(G):
                    b = G * g + j
                    cq = conv_q[g]
                    jj = j
                    for c in range(CH):
                        nc.tensor.matmul(
                            va_ps[:, c, b:b + 1],
                            lhsT=cq[0:LC + 1, jj, 0, c * 128:(c + 1) * 128],
                            rhs=pi_cols[0:LC + 1, 0, j:j + 1],
                            start=first_va[0], stop=False,
                        )
                        first_va[0] = False
                        last = b == S - 1 and c == CH - 1
                        nc.tensor.matmul(
                            va_ps[:, c, b:b + 1],
                            lhsT=cq[0:LC, jj, 1, c * 128:(c + 1) * 128],
                            rhs=pi_cols[0:LC, 1, j:j + 1],
                            start=False, stop=last,
                        )

            # --- atten_out = visAtt + ho; h = tanh(W_h @ . + b) ------------
            attn = acts.tile([128, CH, S], BF16, tag="attn")
            nc.vector.tensor_add(attn, va_ps, ho_t)
            h_sb = dense("h", attn, ACTF.Tanh, F32, w_h_c)
            nc.sync.dma_start(out=out_d[:, :, :], in_=h_sb)

    return nc


# ---------------------------------------------------------------------------

_NC_CACHE = {}


def _get_nc(stage: int = 9):
    key = ("nc", stage)
    if key not in _NC_CACHE:
        nc = build_nc(stage)
        nc.compile()
        _NC_CACHE[key] = nc
    return _NC_CACHE[key]


F8NP = mybir.dt.np(F8)
BFNP = mybir.dt.np(BF16)


def make_in_maps(inputs):
    def wpack(w, dt):
        # [128, CH, D]: w[p, k, o] = W[o, k*128+p]
        return np.ascontiguousarray(
            w.T.reshape(CH, 128, D).transpose(1, 0, 2).astype(dt)
        )

    shared = {
        "w_ho": wpack(np.asarray(inputs["W_ho"]), BFNP),
        "w_h": wpack(np.asarray(inputs["W_h"]), BFNP),
        "w_hoe": wpack(np.asarray(inputs["W_hoe"]), F8NP),
        "w_fr": wpack(np.asarray(inputs["W_fr"]), F8NP),
        "w_fre": wpack(np.asarray(inputs["W_fre"]), F8NP),
    }
    bias_row = np.stack(
        [np.asarray(inputs[f"b_{n}"]) for n in ("fr", "fre", "ho", "hoe", "h")]
    )  # [5, D]
    shared["bias_row"] = np.ascontiguousarray(bias_row[None].astype(BFNP))
    wa8 = np.zeros((128, CH, 16), F8NP)
    wa8[:, :, 0] = (
        np.asarray(inputs["W_a"]).reshape(CH, 128).T.astype(F8NP)
    )
    shared["wa8"] = wa8

    cfe_all = np.asarray(inputs["conv_feat_embed"])
    conv_all = np.asarray(inputs["conv_feat"])

    in_maps = []
    for i in range(N_CORES):
        sl = slice(i * S, (i + 1) * S)
        m = dict(shared)

        def xpack(x):
            # [128, CH, S]: x[p, k, b] = v[b, k*128+p]
            return np.ascontiguousarray(
                x.T.reshape(CH, 128, S).transpose(1, 0, 2).astype(BFNP)
            )

        m["xfr_T"] = xpack(np.asarray(inputs["fake_region"])[sl])
        m["xho_T"] = xpack(np.asarray(inputs["h_out"])[sl])

        # cfe8[p, b, s, l] = cfe[b, l, s*128+p]
        m["cfe8"] = np.ascontiguousarray(
            cfe_all[sl].transpose(2, 0, 1).reshape(CH, 128, S, L)
            .transpose(1, 2, 0, 3).astype(F8NP)
        )

        conv8 = np.zeros((LC + 1, S, 2, D), F8NP)
        cs = conv_all[sl].astype(F8NP)          # [S, L, D]
        conv8[1:LC + 1, :, 0, :] = cs[:, 0:LC, :].transpose(1, 0, 2)
        conv8[0:LC, :, 1, :] = cs[:, LC:L, :].transpose(1, 0, 2)
        m["conv8"] = conv8
        in_maps.append(m)
    return in_maps


def run(inputs, trace=False, trace_kwargs=None, stage=9):
    nc = _get_nc(stage)
    in_maps = make_in_maps(inputs)
    res = run_bass_kernel_spmd(
        nc, in_maps, core_ids=list(range(N_CORES)), trace=trace,
        **(trace_kwargs or {}),
    )
    shards = [res.results[i]["out"] for i in range(N_CORES)]
    # out[p, c, b] = h[b, c*128+p]
    h = np.concatenate(
        [s.transpose(2, 1, 0).reshape(S, D) for s in shards], axis=0
    ).astype(np.float32)
    return h, res


def kernel(**inputs) -> np.ndarray:
    h, _ = run(inputs, trace=False)
    return h


if __name__ == "__main__":
    nc = build_nc()
    print(f"built ok: {len(nc.inst_map)} instructions")


# revision 36
# speedup vs baseline: 1.1439x; 1.1439x over previous
"""AdaAtt attention block on 8 TRN2 NeuronCores — v2.

Data-parallel over batch (16/core), weights replicated. Rebuilt around
three findings from the v1 trace: DMA was descriptor-generation-bound
(34.7k small packets, MBU 24%), the PE was LDWEIGHTS/instruction-count
bound (868 matmul+ldw pairs), and DVE burned 35us on per-(b,chunk)
broadcast adds.

Changes:
- Every DMA is SBUF-tile-exact with >=2KB contiguous per-partition runs
  (hundreds of descriptors total instead of 34.7k).
- fp8e4 storage for conv_feat, conv_feat_embed, W_fr, W_fre, W_hoe, W_a
  (numpy-emulated rel-err 5.7e-3 vs the 2e-2 budget); W_ho and W_h stay
  bf16 (fp8 there alone costs 3e-2). Mixed fp8xbf16 matmuls are legal on
  the PE, and fp8 stationaries get the fast weight load path.
- Dense layers accumulate all 8 output chunks in ONE psum bank with a
  single start/stop group; bias enters via K=1 ones-trick matmuls so
  each layer needs one activation instruction.
- The hoe broadcast-add is one DVE tensor_tensor per batch (free-dim
  stride-0 broadcast), tanh is one big ACT op per batch writing fp8.
- Scores use DoubleRow fp8 matmuls (K=256 per instruction, 4/batch)
  into a [16,196] psum tile, so softmax runs batched in natural layout
  with no transposes of scores.
- The l=0 fake-region slot is injected into the conv value tile (row 0
  of the first l-chunk) with a tiny SBUF->SBUF DMA, so visAtt picks it
  up for free; softmax/visAtt run per 4-batch group to shrink the tail.
"""

import sys

if "/opt/trn_rl_repo" not in sys.path:
    sys.path.insert(0, "/opt/trn_rl_repo")

import numpy as np

import concourse.bass as bass
import concourse.tile as tile
from concourse import mybir
from concourse import bacc
from concourse.bass_utils import run_bass_kernel_spmd
from concourse.masks import make_identity

# ---------------------------------------------------------------------------

B, L, D = 128, 196, 1024
N_CORES = 8
S = B // N_CORES          # batches per core
CH = D // 128             # 128-wide chunks of D
LC = 98                   # conv l-chunk rows; c0 holds l0-slot + l=1..98
LP = 104                  # ha l-chunk pitch (98 padded so kt-stride is 16n)
G = 4                     # batches per softmax/visAtt group
NG = S // G

F32 = mybir.dt.float32
BF16 = mybir.dt.bfloat16
F8 = mybir.dt.float8e4

ACTF = mybir.ActivationFunctionType
ALU = mybir.AluOpType
DR = mybir.MatmulPerfMode.DoubleRow


def build_nc(stage: int = 9) -> bass.Bass:
    nc = bacc.Bacc()

    def param(name, shape, dt, out=False):
        return nc.declare_dram_parameter(name, list(shape), dt, isOutput=out)

    xfr_d = param("xfr_T", (128, CH, S), BF16)
    xho_d = param("xho_T", (128, CH, S), BF16)
    w_d = {
        "ho": param("w_ho", (128, CH, D), BF16),
        "hoe": param("w_hoe", (128, CH, D), F8),
        "fr": param("w_fr", (128, CH, D), F8),
        "fre": param("w_fre", (128, CH, D), F8),
        "h": param("w_h", (128, CH, D), BF16),
    }
    bias_d = param("bias_row", (1, 5, D), BF16)
    wa_d = param("wa8", (128, CH, 16), F8)
    cfe_d = param("cfe8", (128, S, CH, L), F8)
    conv_d = param("conv8", (LC + 1, S, 2, D), F8)
    out_d = param("out", (128, CH, S), F32, out=True)

    LI = {"fr": 0, "fre": 1, "ho": 2, "hoe": 3, "h": 4}

    with tile.TileContext(nc) as tc:
        with (
            tc.tile_pool(name="singles", bufs=1) as singles,
            tc.tile_pool(name="w16p", bufs=2) as w16p,
            tc.tile_pool(name="w8p", bufs=4) as w8p,
            tc.tile_pool(name="acts", bufs=1) as acts,
            tc.tile_pool(name="cfep", bufs=NG) as cfep,
            tc.tile_pool(name="hap", bufs=NG) as hap,
            tc.tile_pool(name="convp", bufs=NG) as convp,
            tc.tile_pool(name="sumr", bufs=3) as sumr,
            tc.tile_pool(name="pic", bufs=NG) as pic,
            tc.tile_pool(name="mm_ps", bufs=2, space="PSUM") as mm_ps,
            tc.tile_pool(name="aux_ps", bufs=1, space="PSUM") as aux_ps,
            tc.tile_pool(name="big_ps", bufs=1, space="PSUM") as big_ps,
        ):
            # --- constants / small loads -----------------------------------
            xho_t = singles.tile([128, CH, S], BF16)
            nc.sync.dma_start(out=xho_t, in_=xho_d[:, :, :])
            bias_t = singles.tile([1, 5, D], BF16)
            nc.sync.dma_start(out=bias_t, in_=bias_d[:, :, :])

            def wload(lname, wpool, wdt, rings=(None, None)):
                w_c = []
                for kc in range(2):
                    t = wpool.tile([128, 4, D], wdt, tag=f"w_{wdt}")
                    ring = rings[kc] or nc.sync
                    ring.dma_start(
                        out=t, in_=w_d[lname][:, 4 * kc:4 * kc + 4, :]
                    )
                    w_c.append(t)
                return w_c

            w_ho_c = wload("ho", w16p, BF16)
            w_hoe_c = wload("hoe", w8p, F8)

            ones_t = singles.tile([1, S], BF16)
            nc.vector.memset(ones_t, 1.0)
            id_bf = singles.tile([128, 128], BF16)
            make_identity(nc, id_bf)
            id_f32 = singles.tile([128, 128], F32)
            make_identity(nc, id_f32)

            # --- dense layers (W stationary, one psum bank per layer) ------
            def dense(lname, rhs_sb, func, out_dt, w_c):
                ps = mm_ps.tile([128, CH, S], F32, tag="mm")
                li = LI[lname]
                for o in range(CH):
                    nc.tensor.matmul(
                        ps[:, o, :],
                        lhsT=bias_t[0:1, li, o * 128:(o + 1) * 128],
                        rhs=ones_t,
                        start=(o == 0), stop=False,
                        tile_position=(0, 0),
                    )
                for kc in range(2):
                    for k in range(4):
                        for o in range(CH):
                            last = kc == 1 and k == 3 and o == CH - 1
                            nc.tensor.matmul(
                                ps[:, o, :],
                                lhsT=w_c[kc][:, k, o * 128:(o + 1) * 128],
                                rhs=rhs_sb[:, 4 * kc + k, :],
                                start=False, stop=last,
                            )
                out_sb = acts.tile([128, CH, S], out_dt, tag=f"act_{lname}")
                nc.scalar.activation(
                    out=out_sb.rearrange("p c b -> p (c b)"),
                    in_=ps.rearrange("p c b -> p (c b)"),
                    func=func,
                )
                return out_sb

            # --- stream DMAs, all on the compute-free sync hw queue in
            # need order (a blocked trigger then never wedges compute)
            def cfe_load(q):
                t = cfep.tile(
                    [128, G, CH, L], F8, tag="cfe", name=f"cfe_{q}"
                )
                nc.sync.dma_start(out=t, in_=cfe_d[:, G * q:G * q + G, :, :])
                return t

            cfe_q = [cfe_load(0)]
            wa_t = singles.tile([128, CH, 16], F8)
            nc.sync.dma_start(out=wa_t, in_=wa_d[:, :, :])
            xfr_t = singles.tile([128, CH, S], BF16)
            nc.sync.dma_start(out=xfr_t, in_=xfr_d[:, :, :])
            w_fr_c = wload("fr", w8p, F8)
            cfe_q.append(cfe_load(1))
            w_fre_c = wload("fre", w8p, F8)
            cfe_q.append(cfe_load(2))
            cfe_q.append(cfe_load(3))
            conv_q = []
            for q in range(NG):
                t = convp.tile(
                    [LC + 1, G, 2, D], F8, tag="conv", name=f"conv_{q}"
                )
                nc.sync.dma_start(out=t, in_=conv_d[:, G * q:G * q + G, :, :])
                conv_q.append(t)
            w_h_c = wload("h", w16p, BF16)

            ho_t = dense("ho", xho_t, ACTF.Tanh, BF16, w_ho_c)

            conv_q = []
            for q in range(NG):
                t = convp.tile(
                    [LC + 1, G, 2, D], F8, tag="conv", name=f"conv_{q}"
                )
                conv_q.append(t)
            w_h_c = [
                w16p.tile(
                    [128, 4, D], BF16, tag=f"w_{BF16}", name=f"w_h_{kc}"
                )
                for kc in range(2)
            ]
            gate_src = ho_t[0:1, 0, 0:4]
            pool_order = [
                (conv_q[0], conv_d[:, 0:G, :, :]),
                (conv_q[1], conv_d[:, G:2 * G, :, :]),
                (w_h_c[0], w_d["h"][:, 0:4, :]),
                (w_h_c[1], w_d["h"][:, 4:8, :]),
                (conv_q[2], conv_d[:, 2 * G:3 * G, :, :]),
                (conv_q[3], conv_d[:, 3 * G:4 * G, :, :]),
            ]
            for dst, src in pool_order:
                head = (
                    dst[0:1, 0, 0, 0:4] if dst.ndim == 4 else dst[0:1, 0, 0:4]
                )
                nc.gpsimd.tensor_copy(head, gate_src)
                nc.gpsimd.dma_start(out=dst, in_=src)

            hoe_t = dense("hoe", ho_t, ACTF.Identity, BF16, w_hoe_c)

            frst = {}

            def emit_fr_chain():  # kept as a function; called inline below
                # emitted mid score-loop so the ACT/PE queue order matches
                # weight-arrival order (fr/fre ACTs would otherwise wedge
                # the tanh pipeline behind the w_fr DMA)
                fr_t = dense("fr", xfr_t, ACTF.Relu, BF16, w_fr_c)
                fre_t = dense("fre", fr_t, ACTF.Identity, BF16, w_fre_c)

                # --- fr -> natural fp8, inject as l=0 row of conv tiles --------
                frn_ps = aux_ps.tile([16, CH, 128], BF16, tag="sng", bufs=2)
                for c in range(CH):
                    nc.tensor.transpose(frn_ps[:, c, :], fr_t[:, c, :], id_bf)
                fr_nat8 = singles.tile([16, CH, 128], F8)
                nc.scalar.activation(
                    out=fr_nat8.rearrange("b c p -> b (c p)"),
                    in_=frn_ps.rearrange("b c p -> b (c p)"),
                    func=ACTF.Copy,
                )
                for q in range(NG):
                    nc.sync.dma_start(
                        out=conv_q[q][0:1, :, 0, :],
                        in_=fr_nat8[G * q:G * q + G, :, :],
                    )

                # --- l=0 slot score: tanh(fr_e + ho_e) . wa --------------------
                sum0 = acts.tile([128, CH, S], BF16, tag="sum0")
                nc.vector.tensor_add(sum0, fre_t, hoe_t)
                ha0 = acts.tile([128, CH, S], BF16, tag="ha0")
                nc.scalar.activation(
                    out=ha0.rearrange("p c b -> p (c b)"),
                    in_=sum0.rearrange("p c b -> p (c b)"),
                    func=ACTF.Tanh,
                )
                sc0_ps = mm_ps.tile([1, S], F32, tag="mm")
                for c in range(CH):
                    nc.tensor.matmul(
                        sc0_ps,
                        lhsT=wa_t[:, c, 0:1],
                        rhs=ha0[:, c, :],
                        start=(c == 0), stop=(c == CH - 1),
                        tile_position=(0, 0),
                    )
                sc0_sb = singles.tile([1, S], F32)
                nc.vector.tensor_copy(sc0_sb, sc0_ps)
                frst["fr"], frst["fre"] = fr_t, fre_t
                frst["sc0"] = sc0_sb

            emit_fr_chain()

            # --- per-batch: bcast add -> tanh -> DoubleRow score mms -------
            # sc_cols[lp, c, b] = score for l = 1 + c*98 + lp
            sc_cols = big_ps.tile([LC, 2, S], F32, tag="sc")
            ha_q = [
                hap.tile([128, G, CH, 2, LP], F8, tag="ha", name=f"ha_{q}")
                for q in range(NG)
            ]
            GP_B = {4, 8, 12}   # adds offloaded to the idle GpSimd engine
            for b in range(S):
                q, j = b // G, b % G
                eng = nc.gpsimd if b in GP_B else nc.vector
                sum_t = sumr.tile([128, CH, L], BF16, tag="sum", bufs=4)
                eng.tensor_tensor(
                    sum_t,
                    cfe_q[q][:, j, :, :],
                    hoe_t[:, :, b:b + 1].broadcast_to([128, CH, L]),
                    op=ALU.add,
                )
                nc.scalar.activation(
                    out=ha_q[q][:, j, :, :, 0:LC],
                    in_=sum_t.rearrange("p c (two l) -> p c two l", two=2),
                    func=ACTF.Tanh,
                )
                for c in range(2):
                    for sp in range(4):
                        nc.tensor.matmul(
                            sc_cols[:, c, b:b + 1],
                            lhsT=ha_q[q][:, j, 2 * sp:2 * sp + 2, c, 0:LC],
                            rhs=wa_t[:, 2 * sp:2 * sp + 2, 0:1],
                            start=(sp == 0), stop=(sp == 3),
                            perf_mode=DR,
                        )

            # --- per-group: softmax + pi transposes + visAtt ---------------
            va_ps = big_ps.tile([128, CH, S], F32, tag="va")
            first_va = [True]
            for g in range(NG):
                sl = slice(G * g, G * g + G)
                scg = sumr.tile([LC, 2, G], F32, tag="scg", bufs=2)
                nc.scalar.activation(
                    out=scg, in_=sc_cols[:, :, sl], func=ACTF.Copy
                )
                sng = aux_ps.tile([G, 1 + L], F32, tag="sng", bufs=2)
                nc.tensor.transpose(
                    sng[:, 0:1], frst["sc0"][0:1, sl], id_f32[0:1, 0:1]
                )
                for c in range(2):
                    nc.tensor.transpose(
                        sng[:, 1 + c * LC:1 + (c + 1) * LC],
                        scg[:, c, :], id_f32[0:LC, 0:LC],
                    )
                neg_mx = sumr.tile([G, 1], F32, tag="negmx", bufs=2)
                nc.vector.tensor_reduce(
                    out=neg_mx, in_=sng,
                    axis=mybir.AxisListType.X, op=ALU.max, negate=True,
                )
                exp_t = sumr.tile([G, 1 + L], F32, tag="exp", bufs=2)
                nc.scalar.activation(
                    out=exp_t, in_=sng,
                    func=ACTF.Exp, bias=neg_mx, scale=1.0,
                )
                ssum = sumr.tile([G, 1], F32, tag="ssum", bufs=2)
                nc.vector.tensor_reduce(
                    out=ssum, in_=exp_t,
                    axis=mybir.AxisListType.X, op=ALU.add,
                )
                rsum = sumr.tile([G, 1], F32, tag="rsum", bufs=2)
                nc.vector.reciprocal(rsum, ssum)
                pi_n = sumr.tile([G, 1 + L], BF16, tag="pi", bufs=2)
                nc.vector.tensor_scalar_mul(pi_n, exp_t, rsum)

                pi_cols = pic.tile([LC + 1, 2, G], BF16, tag="pic")
                tpg = aux_ps.tile([LC + 1, 2, G], BF16, tag="tpg", bufs=1)
                nc.tensor.transpose(
                    tpg[:, 0, :], pi_n[:, 0:LC + 1], id_bf[0:G, 0:G]
                )
                nc.tensor.transpose(
                    tpg[0:LC, 1, :], pi_n[:, LC + 1:1 + L], id_bf[0:G, 0:G]
                )
                nc.vector.tensor_copy(pi_cols[:, 0, :], tpg[:, 0, :])
                nc.vector.tensor_copy(pi_cols[0:LC, 1, :], tpg[0:LC, 1, :])

                for j in range(G):
                    b = G * g + j
                    cq = conv_q[g]
                    jj = j
                    for c in range(CH):
                        nc.tensor.matmul(
                            va_ps[:, c, b:b + 1],
                            lhsT=cq[0:LC + 1, jj, 0, c * 128:(c + 1) * 128],
                            rhs=pi_cols[0:LC + 1, 0, j:j + 1],
                            start=first_va[0], stop=False,
                        )
                        first_va[0] = False
                        last = b == S - 1 and c == CH - 1
                        nc.tensor.matmul(
                            va_ps[:, c, b:b + 1],
                            lhsT=cq[0:LC, jj, 1, c * 128:(c + 1) * 128],
                            rhs=pi_cols[0:LC, 1, j:j + 1],
                            start=False, stop=last,
                        )

            # --- atten_out = visAtt + ho; h = tanh(W_h @ . + b) ------------
            attn = acts.tile([128, CH, S], BF16, tag="attn")
            nc.vector.tensor_add(attn, va_ps, ho_t)
            h_sb = dense("h", attn, ACTF.Tanh, F32, w_h_c)
            nc.sync.dma_start(out=out_d[:, :, :], in_=h_sb)

    return nc


# ---------------------------------------------------------------------------

_NC_CACHE = {}


def _get_nc(stage: int = 9):
    key = ("nc", stage)
    if key not in _NC_CACHE:
        nc = build_nc(stage)
        nc.compile()
        _NC_CACHE[key] = nc
    return _NC_CACHE[key]


F8NP = mybir.dt.np(F8)
BFNP = mybir.dt.np(BF16)


def make_in_maps(inputs):
    def wpack(w, dt):
        # [128, CH, D]: w[p, k, o] = W[o, k*128+p]
        return np.ascontiguousarray(
            w.T.reshape(CH, 128, D).transpose(1, 0, 2).astype(dt)
        )

    shared = {
        "w_ho": wpack(np.asarray(inputs["W_ho"]), BFNP),
        "w_h": wpack(np.asarray(inputs["W_h"]), BFNP),
        "w_hoe": wpack(np.asarray(inputs["W_hoe"]), F8NP),
        "w_fr": wpack(np.asarray(inputs["W_fr"]), F8NP),
        "w_fre": wpack(np.asarray(inputs["W_fre"]), F8NP),
    }
    bias_row = np.stack(
        [np.asarray(inputs[f"b_{n}"]) for n in ("fr", "fre", "ho", "hoe", "h")]
    )  # [5, D]
    shared["bias_row"] = np.ascontiguousarray(bias_row[None].astype(BFNP))
    wa8 = np.zeros((128, CH, 16), F8NP)
    wa8[:, :, 0] = (
        np.asarray(inputs["W_a"]).reshape(CH, 128).T.astype(F8NP)
    )
    shared["wa8"] = wa8

    cfe_all = np.asarray(inputs["conv_feat_embed"])
    conv_all = np.asarray(inputs["conv_feat"])

    in_maps = []
    for i in range(N_CORES):
        sl = slice(i * S, (i + 1) * S)
        m = dict(shared)

        def xpack(x):
            # [128, CH, S]: x[p, k, b] = v[b, k*128+p]
            return np.ascontiguousarray(
                x.T.reshape(CH, 128, S).transpose(1, 0, 2).astype(BFNP)
            )

        m["xfr_T"] = xpack(np.asarray(inputs["fake_region"])[sl])
        m["xho_T"] = xpack(np.asarray(inputs["h_out"])[sl])

        # cfe8[p, b, s, l] = cfe[b, l, s*128+p]
        m["cfe8"] = np.ascontiguousarray(
            cfe_all[sl].transpose(2, 0, 1).reshape(CH, 128, S, L)
            .transpose(1, 2, 0, 3).astype(F8NP)
        )

        conv8 = np.zeros((LC + 1, S, 2, D), F8NP)
        cs = conv_all[sl].astype(F8NP)          # [S, L, D]
        conv8[1:LC + 1, :, 0, :] = cs[:, 0:LC, :].transpose(1, 0, 2)
        conv8[0:LC, :, 1, :] = cs[:, LC:L, :].transpose(1, 0, 2)
        m["conv8"] = conv8
        in_maps.append(m)
    return in_maps


def run(inputs, trace=False, trace_kwargs=None, stage=9):
    nc = _get_nc(stage)
    in_maps = make_in_maps(inputs)
    res = run_bass_kernel_spmd(
        nc, in_maps, core_ids=list(range(N_CORES)), trace=trace,
        **(trace_kwargs or {}),
    )
    shards = [res.results[i]["out"] for i in range(N_CORES)]
    # out[p, c, b] = h[b, c*128+p]
    h = np.concatenate(
        [s.transpose(2, 1, 0).reshape(S, D) for s in shards], axis=0
    ).astype(np.float32)
    return h, res


def kernel(**inputs) -> np.ndarray:
    h, _ = run(inputs, trace=False)
    return h


if __name__ == "__main__":
    nc = build_nc()
    print(f"built ok: {len(nc.inst_map)} instructions")
